# revision 24
# baseline (speedup 1.0000x reference)
"""Trainium2 Bass kernel for nn_CrossAttentionBlock (B=8, N=1024, C=768, H=12).

Sharding: data-parallel over the batch dim — each of the 8 NeuronCores runs the
full cross-attention block for one batch element. No collectives.

Host marshaling (layout prep, not compute): activations/weights pre-transposed
to feature-major and pre-cast to fp8e4m3 for the projection matmuls; the
out-proj bias is pre-folded into the bf16 residual.

Per-core dataflow, balanced across all four compute engines:
  PE   : QKV projections + out-proj as fp8 DoubleRow matmuls (two 128-feature
         k-blocks per pass, 0.5 cyc/row); attention scores bf16 into S^T[k,q]
         PSUM; attn@V token-major as fp8 DoubleRow with E as stationary and a
         ones-augmented V as moving — O[q, d+1] accumulates both the context
         sum and the softmax denominator; AO transposed back to feature-major
         via is_transpose matmuls; residual added into the out-proj PSUM via
         an identity-lhsT bf16 matmul.
  ACT  : the 96 softmax exp evacuations (S PSUM -> E fp8), sqrt, and the
         LayerNorm (x-mu)*rsigma apply (per-partition scale/bias operands).
  DVE  : projection bias-add evacuations, per-token 1/rowsum reciprocal
         (free-size 8!), O normalize into fp8 (broadcast_to AP), transposed-AO
         PSUM->SBUF copies, bn_stats/bn_aggr.
  Pool : gamma/beta apply (PSUM is off-limits to GPSIMD on this target).

PSUM (8 banks): rotating [128,1024] pair (pv/S/Y, 4) + pj [128,512] (1) +
O [128,8,65] (2) + AOt fp8 [64,1024] (1). Q/K projection blocks for head-pair
k+1 are emitted inside the attention window of pair k so the PE never starves
while ACT (the bottleneck, ~8.3us/head of exp) streams.
"""

import json

import ml_dtypes
import numpy as np

import concourse.bass as bass
import concourse.mybir as mybir
import concourse.tile as tile
from concourse.masks import make_identity

B, N, C, H, D = 8, 1024, 768, 12, 64
KB = C // 128  # feature-dim 128-blocks (6)
TB = N // 128  # token-dim 128-blocks (8)
KP = KB // 2   # DoubleRow k-block pairs (3)
SCALE = D ** -0.5
EPS = 1e-5
F32 = mybir.dt.float32
BF16 = mybir.dt.bfloat16
FP8 = mybir.dt.float8e4
AF = mybir.ActivationFunctionType
ALU = mybir.AluOpType
DR = mybir.MatmulPerfMode.DoubleRow
BF16_NP = ml_dtypes.bfloat16
FP8_NP = ml_dtypes.float8_e4m3

# ---------------------------------------------------------------------------
# Workaround: this walrus build rejects instructions with more than one
# semaphore wait ("Too many sync wait commands").  Legalize the BIR by hoisting
# excess waits onto same-engine NoOps inserted right before the instruction.
# ---------------------------------------------------------------------------
_MAX_WAITS = 1
_legal_counter = [0]


def _legalize_waits(bir_json: bytes) -> bytes:
    m = json.loads(bir_json)
    changed = False
    for fn in m.get("functions", []):
        for bb in fn.get("blocks", []):
            out = []
            for inst in bb.get("instructions", []):
                si = inst.get("sync_info") or {}
                waits = si.get("on_wait") or []
                if len(waits) > _MAX_WAITS:
                    changed = True
                    extra = waits[_MAX_WAITS:]
                    si["on_wait"] = waits[:_MAX_WAITS]
                    for i in range(0, len(extra), _MAX_WAITS):
                        _legal_counter[0] += 1
                        nop = {
                            "engine": inst["engine"],
                            "ins": [],
                            "name": f"I-legalw-{_legal_counter[0]}",
                            "opcode": "NoOp",
                            "outs": [],
                            "sync_info": {
                                "on_update": [],
                                "on_wait": extra[i : i + _MAX_WAITS],
                            },
                        }
                        if "debug" in inst:
                            nop["debug"] = inst["debug"]
                        out.append(nop)
                out.append(inst)
            bb["instructions"] = out
    return json.dumps(m).encode() if changed else bir_json


_hooked = False


def _install_compile_hook():
    global _hooked
    if _hooked:
        return
    _hooked = True
    import concourse.bass_utils as bu

    orig = bu.compile_bir_kernel

    def compile_bir_kernel(bir_json, tmpdir, neff_name="file.neff"):
        return orig(_legalize_waits(bir_json), tmpdir, neff_name)

    bu.compile_bir_kernel = compile_bir_kernel
    try:
        import concourse.bass2jax as b2j

        b2j.compile_bir_kernel = compile_bir_kernel
    except ImportError:
        pass


# ---------------------------------------------------------------------------
# Kernel builder
# ---------------------------------------------------------------------------

def _dram_ap(t, offset, ap):
    return bass.AP(t, offset, ap)


def build_nc(trivial_affine: bool = False) -> bass.Bass:
    nc = bass.Bass()

    q_bf_d = nc.dram_tensor("q_bf", [N, C], BF16, kind="ExternalInput")
    qT8_d = nc.dram_tensor("qT8", [C, N], FP8, kind="ExternalInput")
    cT8_d = nc.dram_tensor("cT8", [C, N], FP8, kind="ExternalInput")
    Wq8_d = nc.dram_tensor("Wq8", [C, C], FP8, kind="ExternalInput")
    Wk8_d = nc.dram_tensor("Wk8", [C, C], FP8, kind="ExternalInput")
    Wv8_d = nc.dram_tensor("Wv8", [C, C], FP8, kind="ExternalInput")
    Wo8_d = nc.dram_tensor("Wo8", [C, C], FP8, kind="ExternalInput")
    bq = nc.dram_tensor("bq", [C], F32, kind="ExternalInput")
    bk = nc.dram_tensor("bk", [C], F32, kind="ExternalInput")
    bv = nc.dram_tensor("bv", [C], F32, kind="ExternalInput")
    gamma = nc.dram_tensor("ln_gamma", [C], F32, kind="ExternalInput")
    beta = nc.dram_tensor("ln_beta", [C], F32, kind="ExternalInput")
    out_t = nc.dram_tensor("out", [N, C], F32, kind="ExternalOutput")

    with tile.TileContext(nc) as tc, nc.allow_low_precision("fp8/bf16 pipeline"):
        _body(tc, nc, q_bf_d, (qT8_d, cT8_d), (Wq8_d, Wk8_d, Wv8_d, Wo8_d),
              (bq, bk, bv), gamma, beta, out_t, trivial_affine)
    return nc


def _body(tc, nc, q_bf_d, actTs, Ws, bs, gamma, beta, out_t, trivial_affine):
    qT8_d, cT8_d = actTs
    Wq8_d, Wk8_d, Wv8_d, Wo8_d = Ws
    bq, bk, bv = bs

    with (
        tc.tile_pool(name="singles", bufs=1) as singles,
        tc.tile_pool(name="feat", bufs=1) as feat,
    ):
        # ---- DMA order: only what head-0 scores need comes first --------
        bq_sb = singles.tile([128, KB], F32, name="bq_sb")
        nc.sync.dma_start(out=bq_sb, in_=_dram_ap(bq, 0, [[1, 128], [128, KB]]))
        bk_sb = singles.tile([128, KB], F32, name="bk_sb")
        nc.sync.dma_start(out=bk_sb, in_=_dram_ap(bk, 0, [[1, 128], [128, KB]]))
        qT8 = feat.tile([128, KB, N], FP8, name="qT8")
        nc.sync.dma_start(
            out=qT8, in_=_dram_ap(qT8_d, 0, [[N, 128], [128 * N, KB], [1, N]])
        )
        Wq8 = feat.tile([128, KB, C], FP8, name="Wq8")
        Wk8 = feat.tile([128, KB, C], FP8, name="Wk8")
        Wv8 = feat.tile([128, KB, C], FP8, name="Wv8")
        Wo8 = feat.tile([128, KB, C], FP8, name="Wo8")
        nc.sync.dma_start(
            out=Wq8, in_=_dram_ap(Wq8_d, 0, [[C, 128], [128 * C, KB], [1, C]])
        )
        cT8 = feat.tile([128, KB, N], FP8, name="cT8")
        nc.sync.dma_start(
            out=cT8, in_=_dram_ap(cT8_d, 0, [[N, 128], [128 * N, KB], [1, N]])
        )
        nc.sync.dma_start(
            out=Wk8, in_=_dram_ap(Wk8_d, 0, [[C, 128], [128 * C, KB], [1, C]])
        )
        # needed from the V-projection / epilogue onwards — queued after
        nc.sync.dma_start(
            out=Wv8, in_=_dram_ap(Wv8_d, 0, [[C, 128], [128 * C, KB], [1, C]])
        )
        bv_bc = singles.tile([128, C], F32, name="bv_bc")
        nc.sync.dma_start(out=bv_bc, in_=_dram_ap(bv, 0, [[0, 128], [1, C]]))
        nc.sync.dma_start(
            out=Wo8, in_=_dram_ap(Wo8_d, 0, [[C, 128], [128 * C, KB], [1, C]])
        )
        q_bf = feat.tile([128, TB, C], BF16, name="q_bf")
        nc.sync.dma_start(
            out=q_bf, in_=_dram_ap(q_bf_d, 0, [[C, 128], [128 * C, TB], [1, C]])
        )
        gamma_bc = singles.tile([128, C], F32, name="gamma_bc")
        nc.sync.dma_start(out=gamma_bc, in_=_dram_ap(gamma, 0, [[0, 128], [1, C]]))
        beta_bc = singles.tile([128, C], F32, name="beta_bc")
        nc.sync.dma_start(out=beta_bc, in_=_dram_ap(beta, 0, [[0, 128], [1, C]]))
        eps_t = singles.tile([128, 1], F32, name="eps_t")
        nc.vector.memset(eps_t, EPS)
        ident_bf = singles.tile([128, 128], BF16, name="ident_bf")
        make_identity(nc, ident_bf)

        QTs = feat.tile([128, KB, N], BF16, name="QTs")
        KTs = feat.tile([128, KB, N], BF16, name="KTs")
        V_aug = feat.tile([128, TB, H, D + 1], FP8, name="V_aug")
        nc.gpsimd.memset(V_aug[:, :, :, D : D + 1], 1.0)
        AO = feat.tile([128, KB, N], FP8, name="AO")

        with (
            tc.tile_pool(name="psS", bufs=1, space="PSUM") as psS,
            tc.tile_pool(name="psP", bufs=1, space="PSUM") as psP,
            tc.tile_pool(name="psO", bufs=1, space="PSUM") as psO,
            tc.tile_pool(name="psT", bufs=1, space="PSUM") as psT,
            tc.tile_pool(name="attn", bufs=1) as attn,
            tc.tile_pool(name="epi", bufs=1) as epi,
        ):
            # -- fp8 DoubleRow Q/K projection of one 128-feature block ----
            def proj_block(wT, srcT, b_sb, dstT, nb):
                for qh in range(2):  # q-halves of 512 tokens
                    pj = psP.tile([128, 512], F32, name="pj", tag="pj", bufs=1)
                    q0 = qh * 512
                    for p in range(KP):
                        nc.tensor.matmul(
                            pj,
                            wT[:, 2 * p : 2 * p + 2, nb * 128 : (nb + 1) * 128],
                            srcT[:, 2 * p : 2 * p + 2, q0 : q0 + 512],
                            start=(p == 0), stop=(p == KP - 1),
                            perf_mode=DR,
                        )
                    nc.vector.tensor_scalar(
                        out=dstT[:, nb, q0 : q0 + 512], in0=pj,
                        scalar1=b_sb[:, nb : nb + 1], scalar2=None, op0=ALU.add,
                    )

            # -- V projection: token-major [128 tok, C/2] halves + fp8 evac.
            # Lives in the pj pool so it never displaces the S rotation.
            def v_block(tb):
                for vh in range(2):
                    c0 = vh * 384
                    pv = psP.tile([128, 384], F32, name="pv", tag="pj", bufs=1)
                    for p in range(KP):
                        nc.tensor.matmul(
                            pv,
                            cT8[:, 2 * p : 2 * p + 2, tb * 128 : (tb + 1) * 128],
                            Wv8[:, 2 * p : 2 * p + 2, c0 : c0 + 384],
                            start=(p == 0), stop=(p == KP - 1),
                            perf_mode=DR,
                        )
                    nc.vector.tensor_add(
                        out=V_aug[:, tb, 6 * vh : 6 * vh + 6, 0:D],
                        in0=pv.rearrange("p (h d) -> p h d", h=6),
                        in1=bv_bc[:, c0 : c0 + 384].rearrange(
                            "p (h d) -> p h d", h=6
                        ),
                    )

            # -- scores + exp stream for one head -------------------------
            def scores_exp(h):
                kbh = h // 2
                ro = D * (h % 2)
                E_full = attn.tile([128, TB, N], FP8, name="E_full",
                                   tag="E_full", bufs=4)
                for kt in range(TB):
                    S = psS.tile([128, N], F32, name="S", tag="s", bufs=2)
                    lhsT = KTs[ro : ro + D, kbh, kt * 128 : (kt + 1) * 128]
                    for ch in range(2):
                        nc.tensor.matmul(
                            S[:, ch * 512 : (ch + 1) * 512],
                            lhsT,
                            QTs[ro : ro + D, kbh, ch * 512 : (ch + 1) * 512],
                            start=True, stop=True,
                        )
                    nc.scalar.activation(
                        out=E_full[:, kt, :], in_=S, func=AF.Exp, scale=SCALE
                    )
                return E_full

            # -- attn@V + normalize + transpose for one head --------------
            def attn_tail(h, E_full):
                kbh = h // 2
                ro = D * (h % 2)
                # per-qb stride padded to 128 fp32 so no matmul out crosses
                # a PSUM bank boundary (still 4KB = 2 banks); qb-outer so
                # each bank has only one open accumulation group at a time
                O = psO.tile([128, TB, 128], F32, name="O", tag="O", bufs=1)
                for qb in range(TB):  # 128-token q blocks
                    for kp in range(4):
                        nc.tensor.matmul(
                            O[:, qb, 0 : D + 1],
                            E_full[:, 2 * kp : 2 * kp + 2,
                                   qb * 128 : (qb + 1) * 128],
                            V_aug[:, 2 * kp : 2 * kp + 2, h, :],
                            start=(kp == 0), stop=(kp == 3),
                            perf_mode=DR,
                        )
                # normalize per-token (partition) and restore feature-major
                rs8 = attn.tile([128, TB], F32, name="rs8", tag="rs8", bufs=2)
                nc.vector.reciprocal(out=rs8, in_=O[:, :, D])
                AO_tok = attn.tile([128, TB, D], BF16, name="AO_tok",
                                   tag="AO_tok", bufs=2)
                nc.vector.tensor_mul(
                    out=AO_tok, in0=O[:, :, 0:D],
                    in1=rs8.broadcast_to([128, TB, D]),
                )
                AOt = psT.tile([D, N], BF16, name="AOt", tag="AOt", bufs=1)
                for qb in range(TB):
                    nc.tensor.transpose(
                        AOt[:, qb * 128 : (qb + 1) * 128], AO_tok[:, qb, :],
                        ident_bf,
                    )
                nc.vector.tensor_copy(out=AO[ro : ro + D, kbh, :], in_=AOt)

            # ---- emission schedule --------------------------------------
            # Each head's attn@V tail is deferred until after the NEXT
            # head's scores/exps are queued, so the ACT exp stream never
            # waits on PE tail work at head boundaries.  The V projection
            # and later Q/K blocks ride inside head windows (PE slack).
            proj_block(Wq8, qT8, bq_sb, QTs, 0)
            proj_block(Wk8, cT8, bk_sb, KTs, 0)
            # per-window extra PE work, balanced so no window exceeds the
            # ACT exp budget: V blocks + next Q/K blocks early, attn tails
            # (1-2 per window) once V is complete
            tails_in_window = {3: 1, 4: 1, 5: 1, 6: 2, 7: 1, 8: 2, 9: 1,
                               10: 2, 11: 1}
            projs_in_window = {0: [("q", 1)], 1: [("k", 1)], 2: [("q", 2)],
                               3: [("k", 2)], 4: [("q", 3)], 5: [("k", 3)],
                               6: [("q", 4)], 7: [("k", 4)], 8: [("q", 5)],
                               9: [("k", 5)]}
            pending = []
            for h in range(H):
                pending.append((h, scores_exp(h)))
                if h < 4:  # two V-projection blocks per early window
                    v_block(2 * h)
                    v_block(2 * h + 1)
                for _ in range(tails_in_window.get(h, 0)):
                    attn_tail(*pending.pop(0))
                for kind, nb in projs_in_window.get(h, []):
                    if kind == "q":
                        proj_block(Wq8, qT8, bq_sb, QTs, nb)
                    else:
                        proj_block(Wk8, cT8, bk_sb, KTs, nb)
            for p in pending:
                attn_tail(*p)

            # ---- out-proj (fp8 DoubleRow) + residual + LayerNorm --------
            # Software-pipelined by one tb so the in-order DVE queue never
            # head-of-line blocks on the ACT sqrt round trip.
            def y_and_stats(tb):
                # alternate PSUM pools for ~4 Y buffers of pipeline depth
                if tb % 2 == 0:
                    Y = psS.tile([128, C], F32, name="Y", tag="s", bufs=2)
                else:
                    Y = psO.tile([128, C], F32, name="Y", tag="O", bufs=1)
                # residual (query + bo, bf16) seeds the accumulator
                for c0, c1 in ((0, 512), (512, C)):
                    nc.tensor.matmul(
                        Y[:, c0:c1], ident_bf, q_bf[:, tb, c0:c1],
                        start=True, stop=False, skip_group_check=True,
                    )
                for p in range(KP):
                    for c0, c1 in ((0, 512), (512, C)):
                        nc.tensor.matmul(
                            Y[:, c0:c1],
                            AO[:, 2 * p : 2 * p + 2, tb * 128 : (tb + 1) * 128],
                            Wo8[:, 2 * p : 2 * p + 2, c0:c1],
                            start=False, stop=(p == KP - 1),
                            perf_mode=DR, skip_group_check=True,
                        )
                stats = epi.tile([128, 3, 6], F32, name="stats", tag="st", bufs=4)
                yv3 = Y.rearrange("p (s q) -> p s q", s=3)
                for s3 in range(3):
                    nc.vector.bn_stats(out=stats[:, s3, :], in_=yv3[:, s3, :])
                mv = epi.tile([128, 2], F32, name="mv", tag="mv", bufs=4)
                nc.vector.bn_aggr(out=mv, in_=stats)
                sd = epi.tile([128, 1], F32, name="sd", tag="sd", bufs=4)
                nc.scalar.activation(
                    out=sd, in_=mv[:, 1:2], func=AF.Sqrt,
                    bias=eps_t[:, 0:1], scale=1.0,
                )
                return Y, mv, sd

            def ln_apply(tb, Y, mv, sd):
                rs = epi.tile([128, 1], F32, name="rs", tag="rs", bufs=4)
                nc.vector.reciprocal(out=rs, in_=sd)
                nm = epi.tile([128, 1], F32, name="nm", tag="nm", bufs=4)
                nc.vector.scalar_tensor_tensor(
                    out=nm, in0=mv[:, 0:1], scalar=-1.0, in1=rs,
                    op0=ALU.mult, op1=ALU.mult,
                )
                xn = epi.tile([128, C], F32, name="xn", tag="xn", bufs=4)
                nc.scalar.activation(
                    out=xn, in_=Y, func=AF.Identity,
                    bias=nm[:, 0:1], scale=rs[:, 0:1],
                )
                if trivial_affine:
                    out_src = xn
                else:
                    yv = epi.tile([128, C], F32, name="yv", tag="yv", bufs=4)
                    nc.gpsimd.tensor_mul(out=yv, in0=xn, in1=gamma_bc)
                    nc.gpsimd.tensor_add(out=yv, in0=yv, in1=beta_bc)
                    out_src = yv
                nc.sync.dma_start(
                    out=_dram_ap(out_t, tb * 128 * C, [[C, 128], [1, C]]),
                    in_=out_src,
                )

            prev = None
            for tb in range(TB):
                cur = (tb, *y_and_stats(tb))
                if prev is not None:
                    ln_apply(*prev)
                prev = cur
            ln_apply(*prev)


# ---------------------------------------------------------------------------
# Entry point
# ---------------------------------------------------------------------------
_nc_cache = {}


def _get_nc(trivial_affine: bool = False):
    if trivial_affine not in _nc_cache:
        _install_compile_hook()
        _nc_cache[trivial_affine] = build_nc(trivial_affine)
    return _nc_cache[trivial_affine]


def make_in_maps(inputs: dict) -> list:
    """Host-side marshaling: shard over batch, pre-transpose to feature-major,
    pre-cast matmul operands to fp8e4m3, fold bo into the bf16 residual."""
    arrs = {k: np.asarray(v, dtype=np.float32) for k, v in inputs.items()}
    shared = {
        "Wq8": np.ascontiguousarray(arrs["Wq"].T.astype(FP8_NP)),
        "Wk8": np.ascontiguousarray(arrs["Wk"].T.astype(FP8_NP)),
        "Wv8": np.ascontiguousarray(arrs["Wv"].T.astype(FP8_NP)),
        "Wo8": np.ascontiguousarray(arrs["Wo"].T.astype(FP8_NP)),
        "bq": arrs["bq"], "bk": arrs["bk"], "bv": arrs["bv"],
        "ln_gamma": arrs["ln_gamma"], "ln_beta": arrs["ln_beta"],
    }
    in_maps = []
    for b in range(B):
        m = dict(shared)
        m["q_bf"] = np.ascontiguousarray(
            (arrs["query"][b] + arrs["bo"]).astype(BF16_NP)
        )
        m["qT8"] = np.ascontiguousarray(arrs["query"][b].T.astype(FP8_NP))
        m["cT8"] = np.ascontiguousarray(arrs["context"][b].T.astype(FP8_NP))
        in_maps.append(m)
    return in_maps


def kernel(**inputs) -> np.ndarray:
    from concourse.bass_utils import run_bass_kernel_spmd

    trivial = bool(
        np.all(np.asarray(inputs["ln_gamma"]) == 1.0)
        and np.all(np.asarray(inputs["ln_beta"]) == 0.0)
    )
    nc = _get_nc(trivial)
    in_maps = make_in_maps(inputs)
    res = run_bass_kernel_spmd(nc, in_maps, core_ids=list(range(B)))
    return np.stack([r["out"] for r in res.results]).astype(np.float32)


# revision 25
# speedup vs baseline: 1.0891x; 1.0891x over previous
"""Trainium2 Bass kernel for nn_CrossAttentionBlock (B=8, N=1024, C=768, H=12).

Sharding: data-parallel over the batch dim — each of the 8 NeuronCores runs the
full cross-attention block for one batch element. No collectives.

Host marshaling (layout prep, not compute): activations/weights pre-transposed
to feature-major and pre-cast to fp8e4m3 for the projection matmuls; the
out-proj bias is pre-folded into the bf16 residual.

Per-core dataflow, balanced across all four compute engines:
  PE   : QKV projections + out-proj as fp8 DoubleRow matmuls (two 128-feature
         k-blocks per pass, 0.5 cyc/row); attention scores bf16 into S^T[k,q]
         PSUM; attn@V token-major as fp8 DoubleRow with E as stationary and a
         ones-augmented V as moving — O[q, d+1] accumulates both the context
         sum and the softmax denominator; AO transposed back to feature-major
         via is_transpose matmuls; residual added into the out-proj PSUM via
         an identity-lhsT bf16 matmul.
  ACT  : the 96 softmax exp evacuations (S PSUM -> E fp8), sqrt, and the
         LayerNorm (x-mu)*rsigma apply (per-partition scale/bias operands).
  DVE  : projection bias-add evacuations, per-token 1/rowsum reciprocal
         (free-size 8!), O normalize into fp8 (broadcast_to AP), transposed-AO
         PSUM->SBUF copies, bn_stats/bn_aggr.
  Pool : gamma/beta apply (PSUM is off-limits to GPSIMD on this target).

PSUM (8 banks): rotating [128,1024] pair (pv/S/Y, 4) + pj [128,512] (1) +
O [128,8,65] (2) + AOt fp8 [64,1024] (1). Q/K projection blocks for head-pair
k+1 are emitted inside the attention window of pair k so the PE never starves
while ACT (the bottleneck, ~8.3us/head of exp) streams.
"""

import json

import ml_dtypes
import numpy as np

import concourse.bass as bass
import concourse.mybir as mybir
import concourse.tile as tile
from concourse.masks import make_identity

B, N, C, H, D = 8, 1024, 768, 12, 64
KB = C // 128  # feature-dim 128-blocks (6)
TB = N // 128  # token-dim 128-blocks (8)
KP = KB // 2   # DoubleRow k-block pairs (3)
SCALE = D ** -0.5
EPS = 1e-5
F32 = mybir.dt.float32
BF16 = mybir.dt.bfloat16
FP8 = mybir.dt.float8e4
AF = mybir.ActivationFunctionType
ALU = mybir.AluOpType
DR = mybir.MatmulPerfMode.DoubleRow
BF16_NP = ml_dtypes.bfloat16
FP8_NP = ml_dtypes.float8_e4m3

# ---------------------------------------------------------------------------
# Workaround: this walrus build rejects instructions with more than one
# semaphore wait ("Too many sync wait commands").  Legalize the BIR by hoisting
# excess waits onto same-engine NoOps inserted right before the instruction.
# ---------------------------------------------------------------------------
_MAX_WAITS = 1
_legal_counter = [0]


def _legalize_waits(bir_json: bytes) -> bytes:
    m = json.loads(bir_json)
    changed = False
    for fn in m.get("functions", []):
        for bb in fn.get("blocks", []):
            out = []
            for inst in bb.get("instructions", []):
                si = inst.get("sync_info") or {}
                waits = si.get("on_wait") or []
                if len(waits) > _MAX_WAITS:
                    changed = True
                    extra = waits[_MAX_WAITS:]
                    si["on_wait"] = waits[:_MAX_WAITS]
                    for i in range(0, len(extra), _MAX_WAITS):
                        _legal_counter[0] += 1
                        nop = {
                            "engine": inst["engine"],
                            "ins": [],
                            "name": f"I-legalw-{_legal_counter[0]}",
                            "opcode": "NoOp",
                            "outs": [],
                            "sync_info": {
                                "on_update": [],
                                "on_wait": extra[i : i + _MAX_WAITS],
                            },
                        }
                        if "debug" in inst:
                            nop["debug"] = inst["debug"]
                        out.append(nop)
                out.append(inst)
            bb["instructions"] = out
    return json.dumps(m).encode() if changed else bir_json


_hooked = False


def _install_compile_hook():
    global _hooked
    if _hooked:
        return
    _hooked = True
    import concourse.bass_utils as bu

    orig = bu.compile_bir_kernel

    def compile_bir_kernel(bir_json, tmpdir, neff_name="file.neff"):
        return orig(_legalize_waits(bir_json), tmpdir, neff_name)

    bu.compile_bir_kernel = compile_bir_kernel
    try:
        import concourse.bass2jax as b2j

        b2j.compile_bir_kernel = compile_bir_kernel
    except ImportError:
        pass


# ---------------------------------------------------------------------------
# Kernel builder
# ---------------------------------------------------------------------------

def _dram_ap(t, offset, ap):
    return bass.AP(t, offset, ap)


def build_nc(trivial_affine: bool = False) -> bass.Bass:
    nc = bass.Bass()

    q_bf_d = nc.dram_tensor("q_bf", [N, C], BF16, kind="ExternalInput")
    qT8_d = nc.dram_tensor("qT8", [C, N], FP8, kind="ExternalInput")
    cT8_d = nc.dram_tensor("cT8", [C, N], FP8, kind="ExternalInput")
    Wq8_d = nc.dram_tensor("Wq8", [C, C], FP8, kind="ExternalInput")
    Wk8_d = nc.dram_tensor("Wk8", [C, C], FP8, kind="ExternalInput")
    Wv8_d = nc.dram_tensor("Wv8", [C, C], FP8, kind="ExternalInput")
    Wo8_d = nc.dram_tensor("Wo8", [C, C], FP8, kind="ExternalInput")
    bq = nc.dram_tensor("bq", [C], F32, kind="ExternalInput")
    bk = nc.dram_tensor("bk", [C], F32, kind="ExternalInput")
    bv = nc.dram_tensor("bv", [C], F32, kind="ExternalInput")
    gamma = nc.dram_tensor("ln_gamma", [C], F32, kind="ExternalInput")
    beta = nc.dram_tensor("ln_beta", [C], F32, kind="ExternalInput")
    out_t = nc.dram_tensor("out", [N, C], F32, kind="ExternalOutput")

    with tile.TileContext(nc) as tc, nc.allow_low_precision("fp8/bf16 pipeline"):
        _body(tc, nc, q_bf_d, (qT8_d, cT8_d), (Wq8_d, Wk8_d, Wv8_d, Wo8_d),
              (bq, bk, bv), gamma, beta, out_t, trivial_affine)
    return nc


def _body(tc, nc, q_bf_d, actTs, Ws, bs, gamma, beta, out_t, trivial_affine):
    qT8_d, cT8_d = actTs
    Wq8_d, Wk8_d, Wv8_d, Wo8_d = Ws
    bq, bk, bv = bs

    with (
        tc.tile_pool(name="singles", bufs=1) as singles,
        tc.tile_pool(name="feat", bufs=1) as feat,
    ):
        # ---- DMA order: only what head-0 scores need comes first --------
        bq_sb = singles.tile([128, KB], F32, name="bq_sb")
        nc.sync.dma_start(out=bq_sb, in_=_dram_ap(bq, 0, [[1, 128], [128, KB]]))
        bk_sb = singles.tile([128, KB], F32, name="bk_sb")
        nc.sync.dma_start(out=bk_sb, in_=_dram_ap(bk, 0, [[1, 128], [128, KB]]))
        qT8 = feat.tile([128, KB, N], FP8, name="qT8")
        nc.sync.dma_start(
            out=qT8, in_=_dram_ap(qT8_d, 0, [[N, 128], [128 * N, KB], [1, N]])
        )
        Wq8 = feat.tile([128, KB, C], FP8, name="Wq8")
        Wk8 = feat.tile([128, KB, C], FP8, name="Wk8")
        Wv8 = feat.tile([128, KB, C], FP8, name="Wv8")
        Wo8 = feat.tile([128, KB, C], FP8, name="Wo8")
        nc.sync.dma_start(
            out=Wq8, in_=_dram_ap(Wq8_d, 0, [[C, 128], [128 * C, KB], [1, C]])
        )
        cT8 = feat.tile([128, KB, N], FP8, name="cT8")
        nc.sync.dma_start(
            out=cT8, in_=_dram_ap(cT8_d, 0, [[N, 128], [128 * N, KB], [1, N]])
        )
        nc.sync.dma_start(
            out=Wk8, in_=_dram_ap(Wk8_d, 0, [[C, 128], [128 * C, KB], [1, C]])
        )
        # needed from the V-projection / epilogue onwards — queued after
        nc.sync.dma_start(
            out=Wv8, in_=_dram_ap(Wv8_d, 0, [[C, 128], [128 * C, KB], [1, C]])
        )
        bv_bc = singles.tile([128, C], F32, name="bv_bc")
        nc.sync.dma_start(out=bv_bc, in_=_dram_ap(bv, 0, [[0, 128], [1, C]]))
        nc.sync.dma_start(
            out=Wo8, in_=_dram_ap(Wo8_d, 0, [[C, 128], [128 * C, KB], [1, C]])
        )
        q_bf = feat.tile([128, TB, C], BF16, name="q_bf")
        nc.sync.dma_start(
            out=q_bf, in_=_dram_ap(q_bf_d, 0, [[C, 128], [128 * C, TB], [1, C]])
        )
        gamma_bc = singles.tile([128, C], F32, name="gamma_bc")
        nc.sync.dma_start(out=gamma_bc, in_=_dram_ap(gamma, 0, [[0, 128], [1, C]]))
        beta_bc = singles.tile([128, C], F32, name="beta_bc")
        nc.sync.dma_start(out=beta_bc, in_=_dram_ap(beta, 0, [[0, 128], [1, C]]))
        eps_t = singles.tile([128, 1], F32, name="eps_t")
        nc.vector.memset(eps_t, EPS)
        ident_bf = singles.tile([128, 128], BF16, name="ident_bf")
        make_identity(nc, ident_bf)

        QTs = feat.tile([128, KB, N], BF16, name="QTs")
        KTs = feat.tile([128, KB, N], BF16, name="KTs")
        V_aug = feat.tile([128, TB, H, D + 1], FP8, name="V_aug")
        nc.gpsimd.memset(V_aug[:, :, :, D : D + 1], 1.0)
        AO = feat.tile([128, KB, N], FP8, name="AO")

        with (
            tc.tile_pool(name="psS", bufs=1, space="PSUM") as psS,
            tc.tile_pool(name="psP", bufs=1, space="PSUM") as psP,
            tc.tile_pool(name="psO", bufs=1, space="PSUM") as psO,
            tc.tile_pool(name="psT", bufs=1, space="PSUM") as psT,
            tc.tile_pool(name="attn", bufs=1) as attn,
            tc.tile_pool(name="epi", bufs=1) as epi,
        ):
            # -- fp8 DoubleRow Q/K projection of one 128-feature block ----
            def proj_block(wT, srcT, b_sb, dstT, nb):
                for qh in range(2):  # q-halves of 512 tokens
                    pj = psP.tile([128, 512], F32, name="pj", tag="pj", bufs=1)
                    q0 = qh * 512
                    for p in range(KP):
                        nc.tensor.matmul(
                            pj,
                            wT[:, 2 * p : 2 * p + 2, nb * 128 : (nb + 1) * 128],
                            srcT[:, 2 * p : 2 * p + 2, q0 : q0 + 512],
                            start=(p == 0), stop=(p == KP - 1),
                            perf_mode=DR,
                        )
                    nc.vector.tensor_scalar(
                        out=dstT[:, nb, q0 : q0 + 512], in0=pj,
                        scalar1=b_sb[:, nb : nb + 1], scalar2=None, op0=ALU.add,
                    )

            # -- V projection: token-major [128 tok, C/2] halves + fp8 evac.
            # Lives in the pj pool so it never displaces the S rotation.
            def v_block(tb):
                for vh in range(2):
                    c0 = vh * 384
                    pv = psP.tile([128, 384], F32, name="pv", tag="pj", bufs=1)
                    for p in range(KP):
                        nc.tensor.matmul(
                            pv,
                            cT8[:, 2 * p : 2 * p + 2, tb * 128 : (tb + 1) * 128],
                            Wv8[:, 2 * p : 2 * p + 2, c0 : c0 + 384],
                            start=(p == 0), stop=(p == KP - 1),
                            perf_mode=DR,
                        )
                    nc.vector.tensor_add(
                        out=V_aug[:, tb, 6 * vh : 6 * vh + 6, 0:D],
                        in0=pv.rearrange("p (h d) -> p h d", h=6),
                        in1=bv_bc[:, c0 : c0 + 384].rearrange(
                            "p (h d) -> p h d", h=6
                        ),
                    )

            # -- scores + exp stream for one head -------------------------
            def scores_exp(h):
                kbh = h // 2
                ro = D * (h % 2)
                E_full = attn.tile([128, TB, N], FP8, name="E_full",
                                   tag="E_full", bufs=4)
                for kt in range(TB):
                    S = psS.tile([128, N], F32, name="S", tag="s", bufs=2)
                    lhsT = KTs[ro : ro + D, kbh, kt * 128 : (kt + 1) * 128]
                    for ch in range(2):
                        nc.tensor.matmul(
                            S[:, ch * 512 : (ch + 1) * 512],
                            lhsT,
                            QTs[ro : ro + D, kbh, ch * 512 : (ch + 1) * 512],
                            start=True, stop=True,
                        )
                    nc.scalar.activation(
                        out=E_full[:, kt, :], in_=S, func=AF.Exp, scale=SCALE
                    )
                return E_full

            # -- attn@V + normalize + transpose for one head --------------
            def attn_tail(h, E_full):
                kbh = h // 2
                ro = D * (h % 2)
                # per-qb stride padded to 128 fp32 so no matmul out crosses
                # a PSUM bank boundary (still 4KB = 2 banks); qb-outer so
                # each bank has only one open accumulation group at a time
                O = psO.tile([128, TB, 128], F32, name="O", tag="O", bufs=1)
                for qb in range(TB):  # 128-token q blocks
                    for kp in range(4):
                        nc.tensor.matmul(
                            O[:, qb, 0 : D + 1],
                            E_full[:, 2 * kp : 2 * kp + 2,
                                   qb * 128 : (qb + 1) * 128],
                            V_aug[:, 2 * kp : 2 * kp + 2, h, :],
                            start=(kp == 0), stop=(kp == 3),
                            perf_mode=DR,
                        )
                # normalize per-token (partition) and restore feature-major
                rs8 = attn.tile([128, TB], F32, name="rs8", tag="rs8", bufs=2)
                nc.vector.reciprocal(out=rs8, in_=O[:, :, D])
                AO_tok = attn.tile([128, TB, D], BF16, name="AO_tok",
                                   tag="AO_tok", bufs=2)
                nc.vector.tensor_mul(
                    out=AO_tok, in0=O[:, :, 0:D],
                    in1=rs8.broadcast_to([128, TB, D]),
                )
                AOt = psT.tile([D, N], BF16, name="AOt", tag="AOt", bufs=1)
                for qb in range(TB):
                    nc.tensor.transpose(
                        AOt[:, qb * 128 : (qb + 1) * 128], AO_tok[:, qb, :],
                        ident_bf,
                    )
                nc.vector.tensor_copy(out=AO[ro : ro + D, kbh, :], in_=AOt)

            # ---- emission schedule --------------------------------------
            # Each head's attn@V tail is deferred until after the NEXT
            # head's scores/exps are queued, so the ACT exp stream never
            # waits on PE tail work at head boundaries.  The V projection
            # and later Q/K blocks ride inside head windows (PE slack).
            proj_block(Wq8, qT8, bq_sb, QTs, 0)
            proj_block(Wk8, cT8, bk_sb, KTs, 0)
            # per-window extra PE work, balanced so no window exceeds the
            # ACT exp budget: V blocks + next Q/K blocks early, attn tails
            # (1-2 per window) once V is complete
            tails_in_window = {3: 1, 4: 1, 5: 1, 6: 2, 7: 1, 8: 2, 9: 1,
                               10: 2, 11: 1}
            projs_in_window = {0: [("q", 1)], 1: [("k", 1)], 2: [("q", 2)],
                               3: [("k", 2)], 4: [("q", 3)], 5: [("k", 3)],
                               6: [("q", 4)], 7: [("k", 4)], 8: [("q", 5)],
                               9: [("k", 5)]}
            pending = []
            for h in range(H):
                pending.append((h, scores_exp(h)))
                if h < 4:  # two V-projection blocks per early window
                    v_block(2 * h)
                    v_block(2 * h + 1)
                for _ in range(tails_in_window.get(h, 0)):
                    attn_tail(*pending.pop(0))
                for kind, nb in projs_in_window.get(h, []):
                    if kind == "q":
                        proj_block(Wq8, qT8, bq_sb, QTs, nb)
                    else:
                        proj_block(Wk8, cT8, bk_sb, KTs, nb)
            for p in pending:
                attn_tail(*p)

            # ---- out-proj (fp8 DoubleRow) + residual + LayerNorm --------
            # Software-pipelined by one tb so the in-order DVE queue never
            # head-of-line blocks on the ACT sqrt round trip.
            def y_and_stats(tb):
                # alternate PSUM pools for ~4 Y buffers of pipeline depth
                if tb % 2 == 0:
                    Y = psS.tile([128, C], F32, name="Y", tag="s", bufs=2)
                else:
                    Y = psO.tile([128, C], F32, name="Y", tag="O", bufs=1)
                stats = epi.tile([128, 2, 6], F32, name="stats", tag="st", bufs=4)
                # column-chunk-major so bn_stats starts as soon as its
                # chunk's accumulation closes; residual seeds each chunk
                for ci, (c0, c1) in enumerate(((0, 512), (512, C))):
                    nc.tensor.matmul(
                        Y[:, c0:c1], ident_bf, q_bf[:, tb, c0:c1],
                        start=True, stop=False, skip_group_check=True,
                    )
                    for p in range(KP):
                        nc.tensor.matmul(
                            Y[:, c0:c1],
                            AO[:, 2 * p : 2 * p + 2, tb * 128 : (tb + 1) * 128],
                            Wo8[:, 2 * p : 2 * p + 2, c0:c1],
                            start=False, stop=(p == KP - 1),
                            perf_mode=DR, skip_group_check=True,
                        )
                    nc.vector.bn_stats(out=stats[:, ci, :], in_=Y[:, c0:c1])
                mv = epi.tile([128, 2], F32, name="mv", tag="mv", bufs=4)
                nc.vector.bn_aggr(out=mv, in_=stats)
                sd = epi.tile([128, 1], F32, name="sd", tag="sd", bufs=4)
                nc.scalar.activation(
                    out=sd, in_=mv[:, 1:2], func=AF.Sqrt,
                    bias=eps_t[:, 0:1], scale=1.0,
                )
                return Y, mv, sd

            def ln_apply(tb, Y, mv, sd):
                rs = epi.tile([128, 1], F32, name="rs", tag="rs", bufs=4)
                nc.vector.reciprocal(out=rs, in_=sd)
                xn = epi.tile([128, C], F32, name="xn", tag="xn", bufs=4)
                nc.vector.tensor_scalar(
                    out=xn, in0=Y, scalar1=mv[:, 0:1], scalar2=rs,
                    op0=ALU.subtract, op1=ALU.mult,
                )
                if trivial_affine:
                    out_src = xn
                else:
                    yv = epi.tile([128, C], F32, name="yv", tag="yv", bufs=4)
                    nc.gpsimd.tensor_mul(out=yv, in0=xn, in1=gamma_bc)
                    nc.gpsimd.tensor_add(out=yv, in0=yv, in1=beta_bc)
                    out_src = yv
                nc.sync.dma_start(
                    out=_dram_ap(out_t, tb * 128 * C, [[C, 128], [1, C]]),
                    in_=out_src,
                )

            prev = None
            for tb in range(TB):
                cur = (tb, *y_and_stats(tb))
                if prev is not None:
                    ln_apply(*prev)
                prev = cur
            ln_apply(*prev)


# ---------------------------------------------------------------------------
# Entry point
# ---------------------------------------------------------------------------
_nc_cache = {}


def _get_nc(trivial_affine: bool = False):
    if trivial_affine not in _nc_cache:
        _install_compile_hook()
        _nc_cache[trivial_affine] = build_nc(trivial_affine)
    return _nc_cache[trivial_affine]


def make_in_maps(inputs: dict) -> list:
    """Host-side marshaling: shard over batch, pre-transpose to feature-major,
    pre-cast matmul operands to fp8e4m3, fold bo into the bf16 residual."""
    arrs = {k: np.asarray(v, dtype=np.float32) for k, v in inputs.items()}
    shared = {
        "Wq8": np.ascontiguousarray(arrs["Wq"].T.astype(FP8_NP)),
        "Wk8": np.ascontiguousarray(arrs["Wk"].T.astype(FP8_NP)),
        "Wv8": np.ascontiguousarray(arrs["Wv"].T.astype(FP8_NP)),
        "Wo8": np.ascontiguousarray(arrs["Wo"].T.astype(FP8_NP)),
        "bq": arrs["bq"], "bk": arrs["bk"], "bv": arrs["bv"],
        "ln_gamma": arrs["ln_gamma"], "ln_beta": arrs["ln_beta"],
    }
    in_maps = []
    for b in range(B):
        m = dict(shared)
        m["q_bf"] = np.ascontiguousarray(
            (arrs["query"][b] + arrs["bo"]).astype(BF16_NP)
        )
        m["qT8"] = np.ascontiguousarray(arrs["query"][b].T.astype(FP8_NP))
        m["cT8"] = np.ascontiguousarray(arrs["context"][b].T.astype(FP8_NP))
        in_maps.append(m)
    return in_maps


def kernel(**inputs) -> np.ndarray:
    from concourse.bass_utils import run_bass_kernel_spmd

    trivial = bool(
        np.all(np.asarray(inputs["ln_gamma"]) == 1.0)
        and np.all(np.asarray(inputs["ln_beta"]) == 0.0)
    )
    nc = _get_nc(trivial)
    in_maps = make_in_maps(inputs)
    res = run_bass_kernel_spmd(nc, in_maps, core_ids=list(range(B)))
    return np.stack([r["out"] for r in res.results]).astype(np.float32)


# revision 26
# speedup vs baseline: 1.0964x; 1.0067x over previous
"""Trainium2 Bass kernel for nn_CrossAttentionBlock (B=8, N=1024, C=768, H=12).

Sharding: data-parallel over the batch dim — each of the 8 NeuronCores runs the
full cross-attention block for one batch element. No collectives.

Host marshaling (layout prep, not compute): activations/weights pre-transposed
to feature-major and pre-cast to fp8e4m3 for the projection matmuls; the
out-proj bias is pre-folded into the bf16 residual.

Per-core dataflow, balanced across all four compute engines:
  PE   : QKV projections + out-proj as fp8 DoubleRow matmuls (two 128-feature
         k-blocks per pass, 0.5 cyc/row); attention scores bf16 into S^T[k,q]
         PSUM; attn@V token-major as fp8 DoubleRow with E as stationary and a
         ones-augmented V as moving — O[q, d+1] accumulates both the context
         sum and the softmax denominator; AO transposed back to feature-major
         via is_transpose matmuls; residual added into the out-proj PSUM via
         an identity-lhsT bf16 matmul.
  ACT  : the 96 softmax exp evacuations (S PSUM -> E fp8), sqrt, and the
         LayerNorm (x-mu)*rsigma apply (per-partition scale/bias operands).
  DVE  : projection bias-add evacuations, per-token 1/rowsum reciprocal
         (free-size 8!), O normalize into fp8 (broadcast_to AP), transposed-AO
         PSUM->SBUF copies, bn_stats/bn_aggr.
  Pool : gamma/beta apply (PSUM is off-limits to GPSIMD on this target).

PSUM (8 banks): rotating [128,1024] pair (pv/S/Y, 4) + pj [128,512] (1) +
O [128,8,65] (2) + AOt fp8 [64,1024] (1). Q/K projection blocks for head-pair
k+1 are emitted inside the attention window of pair k so the PE never starves
while ACT (the bottleneck, ~8.3us/head of exp) streams.
"""

import json

import ml_dtypes
import numpy as np

import concourse.bass as bass
import concourse.mybir as mybir
import concourse.tile as tile
from concourse.masks import make_identity

B, N, C, H, D = 8, 1024, 768, 12, 64
KB = C // 128  # feature-dim 128-blocks (6)
TB = N // 128  # token-dim 128-blocks (8)
KP = KB // 2   # DoubleRow k-block pairs (3)
SCALE = D ** -0.5
EPS = 1e-5
F32 = mybir.dt.float32
BF16 = mybir.dt.bfloat16
FP8 = mybir.dt.float8e4
AF = mybir.ActivationFunctionType
ALU = mybir.AluOpType
DR = mybir.MatmulPerfMode.DoubleRow
BF16_NP = ml_dtypes.bfloat16
FP8_NP = ml_dtypes.float8_e4m3

# ---------------------------------------------------------------------------
# Workaround: this walrus build rejects instructions with more than one
# semaphore wait ("Too many sync wait commands").  Legalize the BIR by hoisting
# excess waits onto same-engine NoOps inserted right before the instruction.
# ---------------------------------------------------------------------------
_MAX_WAITS = 1
_legal_counter = [0]


def _legalize_waits(bir_json: bytes) -> bytes:
    m = json.loads(bir_json)
    changed = False
    for fn in m.get("functions", []):
        for bb in fn.get("blocks", []):
            out = []
            for inst in bb.get("instructions", []):
                si = inst.get("sync_info") or {}
                waits = si.get("on_wait") or []
                if len(waits) > _MAX_WAITS:
                    changed = True
                    extra = waits[_MAX_WAITS:]
                    si["on_wait"] = waits[:_MAX_WAITS]
                    for i in range(0, len(extra), _MAX_WAITS):
                        _legal_counter[0] += 1
                        nop = {
                            "engine": inst["engine"],
                            "ins": [],
                            "name": f"I-legalw-{_legal_counter[0]}",
                            "opcode": "NoOp",
                            "outs": [],
                            "sync_info": {
                                "on_update": [],
                                "on_wait": extra[i : i + _MAX_WAITS],
                            },
                        }
                        if "debug" in inst:
                            nop["debug"] = inst["debug"]
                        out.append(nop)
                out.append(inst)
            bb["instructions"] = out
    return json.dumps(m).encode() if changed else bir_json


_hooked = False


def _install_compile_hook():
    global _hooked
    if _hooked:
        return
    _hooked = True
    import concourse.bass_utils as bu

    orig = bu.compile_bir_kernel

    def compile_bir_kernel(bir_json, tmpdir, neff_name="file.neff"):
        return orig(_legalize_waits(bir_json), tmpdir, neff_name)

    bu.compile_bir_kernel = compile_bir_kernel
    try:
        import concourse.bass2jax as b2j

        b2j.compile_bir_kernel = compile_bir_kernel
    except ImportError:
        pass


# ---------------------------------------------------------------------------
# Kernel builder
# ---------------------------------------------------------------------------

def _dram_ap(t, offset, ap):
    return bass.AP(t, offset, ap)


def build_nc(trivial_affine: bool = False) -> bass.Bass:
    nc = bass.Bass()

    q_bf_d = nc.dram_tensor("q_bf", [N, C], BF16, kind="ExternalInput")
    qw8_d = nc.dram_tensor("qw8", [C, N + C], FP8, kind="ExternalInput")
    cw8_d = nc.dram_tensor("cw8", [C, N + C], FP8, kind="ExternalInput")
    Wv8_d = nc.dram_tensor("Wv8", [C, C], FP8, kind="ExternalInput")
    Wo8_d = nc.dram_tensor("Wo8", [C, C], FP8, kind="ExternalInput")
    bq = nc.dram_tensor("bq", [C], F32, kind="ExternalInput")
    bk = nc.dram_tensor("bk", [C], F32, kind="ExternalInput")
    bv = nc.dram_tensor("bv", [C], F32, kind="ExternalInput")
    gamma = nc.dram_tensor("ln_gamma", [C], F32, kind="ExternalInput")
    beta = nc.dram_tensor("ln_beta", [C], F32, kind="ExternalInput")
    out_t = nc.dram_tensor("out", [N, C], F32, kind="ExternalOutput")

    with tile.TileContext(nc) as tc, nc.allow_low_precision("fp8/bf16 pipeline"):
        _body(tc, nc, q_bf_d, (qw8_d, cw8_d), (Wv8_d, Wo8_d),
              (bq, bk, bv), gamma, beta, out_t, trivial_affine)
    return nc


def _body(tc, nc, q_bf_d, actTs, Ws, bs, gamma, beta, out_t, trivial_affine):
    qw8_d, cw8_d = actTs
    Wv8_d, Wo8_d = Ws
    bq, bk, bv = bs

    with (
        tc.tile_pool(name="singles", bufs=1) as singles,
        tc.tile_pool(name="feat", bufs=1) as feat,
    ):
        # ---- DMA order: only what head-0 scores need comes first --------
        NC_ = N + C
        qw8 = feat.tile([128, KB, NC_], FP8, name="qw8")
        nc.sync.dma_start(
            out=qw8,
            in_=_dram_ap(qw8_d, 0, [[NC_, 128], [128 * NC_, KB], [1, NC_]]),
        )
        bq_sb = singles.tile([128, KB], F32, name="bq_sb")
        nc.sync.dma_start(out=bq_sb, in_=_dram_ap(bq, 0, [[1, 128], [128, KB]]))
        cw8 = feat.tile([128, KB, NC_], FP8, name="cw8")
        nc.sync.dma_start(
            out=cw8,
            in_=_dram_ap(cw8_d, 0, [[NC_, 128], [128 * NC_, KB], [1, NC_]]),
        )
        bk_sb = singles.tile([128, KB], F32, name="bk_sb")
        nc.sync.dma_start(out=bk_sb, in_=_dram_ap(bk, 0, [[1, 128], [128, KB]]))
        qT8 = qw8[:, :, 0:N]
        Wq8 = qw8[:, :, N:NC_]
        cT8 = cw8[:, :, 0:N]
        Wk8 = cw8[:, :, N:NC_]
        Wv8 = feat.tile([128, KB, C], FP8, name="Wv8")
        Wo8 = feat.tile([128, KB, C], FP8, name="Wo8")
        # needed from the V-projection / epilogue onwards — queued after
        nc.sync.dma_start(
            out=Wv8, in_=_dram_ap(Wv8_d, 0, [[C, 128], [128 * C, KB], [1, C]])
        )
        bv_bc = singles.tile([128, C], F32, name="bv_bc")
        nc.sync.dma_start(out=bv_bc, in_=_dram_ap(bv, 0, [[0, 128], [1, C]]))
        nc.sync.dma_start(
            out=Wo8, in_=_dram_ap(Wo8_d, 0, [[C, 128], [128 * C, KB], [1, C]])
        )
        q_bf = feat.tile([128, TB, C], BF16, name="q_bf")
        nc.sync.dma_start(
            out=q_bf, in_=_dram_ap(q_bf_d, 0, [[C, 128], [128 * C, TB], [1, C]])
        )
        gamma_bc = singles.tile([128, C], F32, name="gamma_bc")
        nc.sync.dma_start(out=gamma_bc, in_=_dram_ap(gamma, 0, [[0, 128], [1, C]]))
        beta_bc = singles.tile([128, C], F32, name="beta_bc")
        nc.sync.dma_start(out=beta_bc, in_=_dram_ap(beta, 0, [[0, 128], [1, C]]))
        eps_t = singles.tile([128, 1], F32, name="eps_t")
        nc.vector.memset(eps_t, EPS)
        ident_bf = singles.tile([128, 128], BF16, name="ident_bf")
        make_identity(nc, ident_bf)

        QTs = feat.tile([128, KB, N], BF16, name="QTs")
        KTs = feat.tile([128, KB, N], BF16, name="KTs")
        V_aug = feat.tile([128, TB, H, D + 1], FP8, name="V_aug")
        nc.gpsimd.memset(V_aug[:, :, :, D : D + 1], 1.0)
        AO = feat.tile([128, KB, N], FP8, name="AO")

        with (
            tc.tile_pool(name="psS", bufs=1, space="PSUM") as psS,
            tc.tile_pool(name="psP", bufs=1, space="PSUM") as psP,
            tc.tile_pool(name="psO", bufs=1, space="PSUM") as psO,
            tc.tile_pool(name="psT", bufs=1, space="PSUM") as psT,
            tc.tile_pool(name="attn", bufs=1) as attn,
            tc.tile_pool(name="epi", bufs=1) as epi,
        ):
            # -- fp8 DoubleRow Q/K projection of one 128-feature block ----
            def proj_block(wT, srcT, b_sb, dstT, nb):
                for qh in range(2):  # q-halves of 512 tokens
                    pj = psP.tile([128, 512], F32, name="pj", tag="pj", bufs=1)
                    q0 = qh * 512
                    for p in range(KP):
                        nc.tensor.matmul(
                            pj,
                            wT[:, 2 * p : 2 * p + 2, nb * 128 : (nb + 1) * 128],
                            srcT[:, 2 * p : 2 * p + 2, q0 : q0 + 512],
                            start=(p == 0), stop=(p == KP - 1),
                            perf_mode=DR,
                        )
                    nc.vector.tensor_scalar(
                        out=dstT[:, nb, q0 : q0 + 512], in0=pj,
                        scalar1=b_sb[:, nb : nb + 1], scalar2=None, op0=ALU.add,
                    )

            # -- V projection: token-major [128 tok, C/2] halves + fp8 evac.
            # Lives in the pj pool so it never displaces the S rotation.
            def v_block(tb):
                for vh in range(2):
                    c0 = vh * 384
                    pv = psP.tile([128, 384], F32, name="pv", tag="pj", bufs=1)
                    for p in range(KP):
                        nc.tensor.matmul(
                            pv,
                            cT8[:, 2 * p : 2 * p + 2, tb * 128 : (tb + 1) * 128],
                            Wv8[:, 2 * p : 2 * p + 2, c0 : c0 + 384],
                            start=(p == 0), stop=(p == KP - 1),
                            perf_mode=DR,
                        )
                    nc.vector.tensor_add(
                        out=V_aug[:, tb, 6 * vh : 6 * vh + 6, 0:D],
                        in0=pv.rearrange("p (h d) -> p h d", h=6),
                        in1=bv_bc[:, c0 : c0 + 384].rearrange(
                            "p (h d) -> p h d", h=6
                        ),
                    )

            # -- scores + exp stream for one head -------------------------
            def scores_exp(h):
                kbh = h // 2
                ro = D * (h % 2)
                E_full = attn.tile([128, TB, N], FP8, name="E_full",
                                   tag="E_full", bufs=4)
                for kt in range(TB):
                    S = psS.tile([128, N], F32, name="S", tag="s", bufs=2)
                    lhsT = KTs[ro : ro + D, kbh, kt * 128 : (kt + 1) * 128]
                    for ch in range(2):
                        nc.tensor.matmul(
                            S[:, ch * 512 : (ch + 1) * 512],
                            lhsT,
                            QTs[ro : ro + D, kbh, ch * 512 : (ch + 1) * 512],
                            start=True, stop=True,
                        )
                    nc.scalar.activation(
                        out=E_full[:, kt, :], in_=S, func=AF.Exp, scale=SCALE
                    )
                return E_full

            # -- attn@V + normalize + transpose for one head --------------
            def attn_tail(h, E_full):
                kbh = h // 2
                ro = D * (h % 2)
                # per-qb stride padded to 128 fp32 so no matmul out crosses
                # a PSUM bank boundary (still 4KB = 2 banks); qb-outer so
                # each bank has only one open accumulation group at a time
                O = psO.tile([128, TB, 128], F32, name="O", tag="O", bufs=1)
                for qb in range(TB):  # 128-token q blocks
                    for kp in range(4):
                        nc.tensor.matmul(
                            O[:, qb, 0 : D + 1],
                            E_full[:, 2 * kp : 2 * kp + 2,
                                   qb * 128 : (qb + 1) * 128],
                            V_aug[:, 2 * kp : 2 * kp + 2, h, :],
                            start=(kp == 0), stop=(kp == 3),
                            perf_mode=DR,
                        )
                # normalize per-token (partition) and restore feature-major
                rs8 = attn.tile([128, TB], F32, name="rs8", tag="rs8", bufs=2)
                nc.vector.reciprocal(out=rs8, in_=O[:, :, D])
                AO_tok = attn.tile([128, TB, D], BF16, name="AO_tok",
                                   tag="AO_tok", bufs=2)
                nc.vector.tensor_mul(
                    out=AO_tok, in0=O[:, :, 0:D],
                    in1=rs8.broadcast_to([128, TB, D]),
                )
                AOt = psT.tile([D, N], BF16, name="AOt", tag="AOt", bufs=1)
                for qb in range(TB):
                    nc.tensor.transpose(
                        AOt[:, qb * 128 : (qb + 1) * 128], AO_tok[:, qb, :],
                        ident_bf,
                    )
                nc.vector.tensor_copy(out=AO[ro : ro + D, kbh, :], in_=AOt)

            # ---- emission schedule --------------------------------------
            # Each head's attn@V tail is deferred until after the NEXT
            # head's scores/exps are queued, so the ACT exp stream never
            # waits on PE tail work at head boundaries.  The V projection
            # and later Q/K blocks ride inside head windows (PE slack).
            proj_block(Wq8, qT8, bq_sb, QTs, 0)
            proj_block(Wk8, cT8, bk_sb, KTs, 0)
            # per-window extra PE work, balanced so no window exceeds the
            # ACT exp budget: V blocks + next Q/K blocks early, attn tails
            # (1-2 per window) once V is complete
            tails_in_window = {3: 1, 4: 1, 5: 1, 6: 2, 7: 1, 8: 2, 9: 1,
                               10: 2, 11: 1}
            projs_in_window = {0: [("q", 1)], 1: [("k", 1)], 2: [("q", 2)],
                               3: [("k", 2)], 4: [("q", 3)], 5: [("k", 3)],
                               6: [("q", 4)], 7: [("k", 4)], 8: [("q", 5)],
                               9: [("k", 5)]}
            pending = []
            for h in range(H):
                pending.append((h, scores_exp(h)))
                if h < 4:  # two V-projection blocks per early window
                    v_block(2 * h)
                    v_block(2 * h + 1)
                for _ in range(tails_in_window.get(h, 0)):
                    attn_tail(*pending.pop(0))
                for kind, nb in projs_in_window.get(h, []):
                    if kind == "q":
                        proj_block(Wq8, qT8, bq_sb, QTs, nb)
                    else:
                        proj_block(Wk8, cT8, bk_sb, KTs, nb)
            for p in pending:
                attn_tail(*p)

            # ---- out-proj (fp8 DoubleRow) + residual + LayerNorm --------
            # Software-pipelined by one tb so the in-order DVE queue never
            # head-of-line blocks on the ACT sqrt round trip.
            def y_and_stats(tb):
                # alternate PSUM pools for ~4 Y buffers of pipeline depth
                if tb % 2 == 0:
                    Y = psS.tile([128, C], F32, name="Y", tag="s", bufs=2)
                else:
                    Y = psO.tile([128, C], F32, name="Y", tag="O", bufs=1)
                stats = epi.tile([128, 2, 6], F32, name="stats", tag="st", bufs=4)
                # column-chunk-major so bn_stats starts as soon as its
                # chunk's accumulation closes; residual seeds each chunk
                for ci, (c0, c1) in enumerate(((0, 512), (512, C))):
                    nc.tensor.matmul(
                        Y[:, c0:c1], ident_bf, q_bf[:, tb, c0:c1],
                        start=True, stop=False, skip_group_check=True,
                    )
                    for p in range(KP):
                        nc.tensor.matmul(
                            Y[:, c0:c1],
                            AO[:, 2 * p : 2 * p + 2, tb * 128 : (tb + 1) * 128],
                            Wo8[:, 2 * p : 2 * p + 2, c0:c1],
                            start=False, stop=(p == KP - 1),
                            perf_mode=DR, skip_group_check=True,
                        )
                    nc.vector.bn_stats(out=stats[:, ci, :], in_=Y[:, c0:c1])
                mv = epi.tile([128, 2], F32, name="mv", tag="mv", bufs=4)
                nc.vector.bn_aggr(out=mv, in_=stats)
                sd = epi.tile([128, 1], F32, name="sd", tag="sd", bufs=4)
                nc.scalar.activation(
                    out=sd, in_=mv[:, 1:2], func=AF.Sqrt,
                    bias=eps_t[:, 0:1], scale=1.0,
                )
                return Y, mv, sd

            def ln_apply(tb, Y, mv, sd):
                rs = epi.tile([128, 1], F32, name="rs", tag="rs", bufs=4)
                nc.vector.reciprocal(out=rs, in_=sd)
                xn = epi.tile([128, C], F32, name="xn", tag="xn", bufs=4)
                nc.vector.tensor_scalar(
                    out=xn, in0=Y, scalar1=mv[:, 0:1], scalar2=rs,
                    op0=ALU.subtract, op1=ALU.mult,
                )
                if trivial_affine:
                    out_src = xn
                else:
                    yv = epi.tile([128, C], F32, name="yv", tag="yv", bufs=4)
                    nc.gpsimd.tensor_mul(out=yv, in0=xn, in1=gamma_bc)
                    nc.gpsimd.tensor_add(out=yv, in0=yv, in1=beta_bc)
                    out_src = yv
                nc.sync.dma_start(
                    out=_dram_ap(out_t, tb * 128 * C, [[C, 128], [1, C]]),
                    in_=out_src,
                )

            prev = None
            for tb in range(TB):
                cur = (tb, *y_and_stats(tb))
                if prev is not None:
                    ln_apply(*prev)
                prev = cur
            ln_apply(*prev)


# ---------------------------------------------------------------------------
# Entry point
# ---------------------------------------------------------------------------
_nc_cache = {}


def _get_nc(trivial_affine: bool = False):
    if trivial_affine not in _nc_cache:
        _install_compile_hook()
        _nc_cache[trivial_affine] = build_nc(trivial_affine)
    return _nc_cache[trivial_affine]


def make_in_maps(inputs: dict) -> list:
    """Host-side marshaling: shard over batch, pre-transpose to feature-major,
    pre-cast matmul operands to fp8e4m3, fold bo into the bf16 residual."""
    arrs = {k: np.asarray(v, dtype=np.float32) for k, v in inputs.items()}
    Wq8 = arrs["Wq"].T.astype(FP8_NP)
    Wk8 = arrs["Wk"].T.astype(FP8_NP)
    shared = {
        "Wv8": np.ascontiguousarray(arrs["Wv"].T.astype(FP8_NP)),
        "Wo8": np.ascontiguousarray(arrs["Wo"].T.astype(FP8_NP)),
        "bq": arrs["bq"], "bk": arrs["bk"], "bv": arrs["bv"],
        "ln_gamma": arrs["ln_gamma"], "ln_beta": arrs["ln_beta"],
    }
    in_maps = []
    for b in range(B):
        m = dict(shared)
        m["q_bf"] = np.ascontiguousarray(
            (arrs["query"][b] + arrs["bo"]).astype(BF16_NP)
        )
        m["qw8"] = np.ascontiguousarray(np.concatenate(
            [arrs["query"][b].T.astype(FP8_NP), Wq8], axis=1))
        m["cw8"] = np.ascontiguousarray(np.concatenate(
            [arrs["context"][b].T.astype(FP8_NP), Wk8], axis=1))
        in_maps.append(m)
    return in_maps


def kernel(**inputs) -> np.ndarray:
    from concourse.bass_utils import run_bass_kernel_spmd

    trivial = bool(
        np.all(np.asarray(inputs["ln_gamma"]) == 1.0)
        and np.all(np.asarray(inputs["ln_beta"]) == 0.0)
    )
    nc = _get_nc(trivial)
    in_maps = make_in_maps(inputs)
    res = run_bass_kernel_spmd(nc, in_maps, core_ids=list(range(B)))
    return np.stack([r["out"] for r in res.results]).astype(np.float32)
